# revision 6
# baseline (speedup 1.0000x reference)
"""Trainium2 Bass kernel for NeighborAggregation.

Math: for x of shape (b, k=1024, c=512) viewed as a 32x32 grid over k,
the reference computes y[cell t] = s(t) * 8^(t-1024) where s is a sum of 4
circularly-shifted neighbors minus 4x, and returns concat(x, y) on the c axis.
8^(t-1024) underflows to exactly 0.0 in fp32 for t <= 974, so y is nonzero
only for the last 49 k-rows (t = 975..1023), whose neighbor cells all live in
grid rows {0, 28..31} = flat cells [0..31] and [896..1023].

Kernel strategy (pure data parallel, batch 64 -> 8 cores x 8 examples):
  1. The bulk x -> out[:, :, 0:512] copy (16.78 MB/core, the dominant cost;
     every byte passes one SDMA engine once at ~25.6 GB/s/engine) is split
     across BOTH HWDGE rings (SP via nc.sync + ACT via nc.scalar), sized so
     the two ring FIFOs drain at the same time given the small transfers that
     share them.
  2. The 49 nonzero y rows per example come from a sparse fp32 matmul with
     the 8^(t-1024) factors folded into the weights. Alternate examples use
     stationary weights zero-padded by 64 leading columns, so their results
     land at PSUM/SBUF partitions 64..112 instead of 0..48. SBUF->HBM store
     descriptors are port-bound (engine = partition//8), so this spreads the
     store work over engines 0-6 AND 8-14 instead of piling on 0-6.
  3. Edge rows (cells 0..31 and 896..1023) and padded weights are uploaded
     pre-transposed/contiguous so loads are few large line-rate descriptors.
  4. The zero region of y is never written: ExternalOutput buffers are
     pre-zeroed by the runner.
"""

from contextlib import ExitStack

import numpy as np

_B_FULL, _K, _C = 64, 1024, 512
_NCORES = 8
_B = _B_FULL // _NCORES  # examples per core
_N = 32
_HI = 896  # first cell of grid rows 28..31
_NNZ = 49  # cells 975..1023 have nonzero factor
_Y0 = _K - _NNZ  # 975
# Per-example partition offset for the y result. A 49-partition DVE write
# must start quad-aligned AND stay within partitions 0-63 or 64-127 (bank
# straddle rule), so only offsets {0, 64} are legal.
_OFFS = [0, 64, 0, 64, 0, 64, 0, 64]
_WCOLS = 64 + _NNZ  # 113 padded stationary columns

# Ring balance: sync ring carries copyA + y stores (49*B descs); scalar ring
# carries the edge/weight loads (~1440 descs) + copyB. Equal ring totals:
_ROWS = _B * _K  # 8192 flat (b, k) rows
_COPY_A = 4620  # rows on the sync ring; rest on scalar

_cached = {}


def _weights():
    """Padded stationary weights, one set per example.

    wp (B, 128, WCOLS) over cells 896..1023 and w2p (B, 32, WCOLS) over cells
    0..31. For example b, column OFFS[b] + o corresponds to output cell
    k = 975 + o; entries are the neighbor coefficients scaled by
    factor[k] = 8^(k-1024) (exact in fp32); all other columns are zero.
    """
    t = np.arange(_K)
    factor = (np.float64(2.0) ** (3.0 * (t - _K))).astype(np.float32)
    w1 = np.zeros((128, _NNZ), np.float32)
    w2 = np.zeros((_N, _NNZ), np.float32)
    for o in range(_NNZ):
        k = _Y0 + o
        i, j = divmod(k, _N)
        f = factor[k]
        i1, i2 = (i + 1) % _N, (i - 2) % _N
        jp, jm = (j + 1) % _N, (j - 2) % _N
        for r, q in [(i1, jp), (i1, jm), (i2, jp), (i2, jm)]:
            cell = _N * r + q
            if cell >= _HI:
                w1[cell - _HI, o] += f
            else:
                w2[cell, o] += f
        w1[k - _HI, o] += np.float32(-4.0) * f
    wp = np.zeros((_B, 128, _WCOLS), np.float32)
    w2p = np.zeros((_B, _N, _WCOLS), np.float32)
    for b in range(_B):
        o = _OFFS[b]
        wp[b, :, o : o + _NNZ] = w1
        w2p[b, :, o : o + _NNZ] = w2
    return wp, w2p


def _build_nc():
    import concourse.bacc as bacc
    import concourse.mybir as mybir
    import concourse.tile as tile

    nc = bacc.Bacc("TRN2", debug=False, num_devices=_NCORES)
    f32 = mybir.dt.float32
    x_ap = nc.dram_tensor("x", (_B, _K, _C), f32, kind="ExternalInput").ap()
    xe1_ap = nc.dram_tensor("xe1", (_B, 128, _C), f32, kind="ExternalInput").ap()
    xe2_ap = nc.dram_tensor("xe2", (_B, _N, _C), f32, kind="ExternalInput").ap()
    wp_ap = nc.dram_tensor("wp", (128, _B * _WCOLS), f32, kind="ExternalInput").ap()
    w2p_ap = nc.dram_tensor("w2p", (_N, _B * _WCOLS), f32, kind="ExternalInput").ap()
    out_ap = nc.dram_tensor("out", (_B, _K, 2 * _C), f32, kind="ExternalOutput").ap()

    x_rows = x_ap.rearrange("b k c -> (b k) c")
    out_rows = out_ap.rearrange("b k c -> (b k) c")

    with tile.TileContext(nc) as tc, ExitStack() as ctx:
        pool = ctx.enter_context(tc.tile_pool(name="sbuf", bufs=1))
        psum_pool = ctx.enter_context(tc.tile_pool(name="psum", bufs=4, space="PSUM"))

        # Sync-ring half of the bulk copy goes first in that ring's FIFO.
        nc.sync.dma_start(
            out=out_rows[0:_COPY_A, 0:_C], in_=x_rows[0:_COPY_A, :]
        )

        # Scalar ring: weight + edge loads first, then its copy half.
        wp = pool.tile([128, _B * _WCOLS], f32, tag="wp")
        nc.scalar.dma_start(out=wp[:], in_=wp_ap)
        w2p = pool.tile([_N, _B * _WCOLS], f32, tag="w2p")
        nc.scalar.dma_start(out=w2p[:], in_=w2p_ap)

        x1s = []
        x2s = []
        for b in range(_B):
            x1 = pool.tile([128, _C], f32, tag=f"x1_{b}", name=f"x1_{b}")
            nc.scalar.dma_start(out=x1[:], in_=xe1_ap[b])
            x1s.append(x1)
            x2 = pool.tile([_N, _C], f32, tag=f"x2_{b}", name=f"x2_{b}")
            nc.scalar.dma_start(out=x2[:], in_=xe2_ap[b])
            x2s.append(x2)

        nc.scalar.dma_start(
            out=out_rows[_COPY_A:_ROWS, 0:_C], in_=x_rows[_COPY_A:_ROWS, :]
        )

        for b in range(_B):
            o = _OFFS[b]
            wsl = slice(b * _WCOLS, (b + 1) * _WCOLS)
            ps = psum_pool.tile([_WCOLS, _C], f32)
            nc.tensor.matmul(ps[:], wp[:, wsl], x1s[b][:], start=True, stop=False)
            nc.tensor.matmul(ps[:], w2p[:, wsl], x2s[b][:], start=False, stop=True)
            y = pool.tile([_WCOLS, _C], f32, tag=f"y_{b}", name=f"y_{b}")
            nc.vector.tensor_copy(y[o : o + _NNZ, :], ps[o : o + _NNZ, :])
            # Store rides the sync ring, behind copyA in its FIFO.
            nc.sync.dma_start(
                out=out_ap[b, _Y0:_K, _C : 2 * _C], in_=y[o : o + _NNZ, :]
            )

    nc.compile()
    return nc


def _get_nc():
    if "nc" not in _cached:
        _cached["nc"] = _build_nc()
    return _cached["nc"]


def _in_maps(x):
    wp, w2p = _weights()
    wpf = np.ascontiguousarray(wp.transpose(1, 0, 2).reshape(128, _B * _WCOLS))
    w2pf = np.ascontiguousarray(w2p.transpose(1, 0, 2).reshape(_N, _B * _WCOLS))
    maps = []
    for i in range(_NCORES):
        xs = np.ascontiguousarray(x[i * _B : (i + 1) * _B])
        maps.append(
            {
                "x": xs,
                "xe1": np.ascontiguousarray(xs[:, _HI:_K, :]),
                "xe2": np.ascontiguousarray(xs[:, 0:_N, :]),
                "wp": wpf,
                "w2p": w2pf,
            }
        )
    return maps


def kernel(x):
    from concourse.bass_utils import run_bass_kernel_spmd

    x = np.asarray(x, dtype=np.float32)
    assert x.shape == (_B_FULL, _K, _C), x.shape
    nc = _get_nc()
    res = run_bass_kernel_spmd(nc, _in_maps(x), list(range(_NCORES)))
    return np.concatenate([r["out"] for r in res.results], axis=0)


# revision 8
# speedup vs baseline: 1.0027x; 1.0027x over previous
"""Trainium2 Bass kernel for NeighborAggregation.

Math: for x of shape (b, k=1024, c=512) viewed as a 32x32 grid over k,
the reference computes y[cell t] = s(t) * 8^(t-1024) where s is a sum of 4
circularly-shifted neighbors minus 4x, and returns concat(x, y) on the c axis.
8^(t-1024) underflows to exactly 0.0 in fp32 for t <= 974, so y is nonzero
only for the last 49 k-rows (t = 975..1023), whose neighbor cells all live in
grid rows {0, 28..31} = flat cells [0..31] and [896..1023].

Kernel strategy (pure data parallel, batch 64 -> 8 cores x 8 examples):
  1. The bulk x -> out[:, :, 0:512] copy (16.78 MB/core, the dominant cost;
     every byte passes one SDMA engine once at ~25.6 GB/s/engine) is split
     across BOTH HWDGE rings (SP via nc.sync + ACT via nc.scalar), sized so
     the two ring FIFOs drain at the same time given the small transfers that
     share them.
  2. The 49 nonzero y rows per example come from a sparse fp32 matmul with
     the 8^(t-1024) factors folded into the weights. Alternate examples use
     stationary weights zero-padded by 64 leading columns, so their results
     land at PSUM/SBUF partitions 64..112 instead of 0..48. SBUF->HBM store
     descriptors are port-bound (engine = partition//8), so this spreads the
     store work over engines 0-6 AND 8-14 instead of piling on 0-6.
  3. Edge rows (cells 0..31 and 896..1023) and padded weights are uploaded
     pre-transposed/contiguous so loads are few large line-rate descriptors.
  4. The zero region of y is never written: ExternalOutput buffers are
     pre-zeroed by the runner.
"""

from contextlib import ExitStack

import numpy as np

_B_FULL, _K, _C = 64, 1024, 512
_NCORES = 8
_B = _B_FULL // _NCORES  # examples per core
_N = 32
_HI = 896  # first cell of grid rows 28..31
_NNZ = 49  # cells 975..1023 have nonzero factor
_Y0 = _K - _NNZ  # 975
# Per-example partition offset for the y result. A 49-partition DVE write
# must start quad-aligned AND stay within partitions 0-63 or 64-127 (bank
# straddle rule), so only offsets {0, 64} are legal.
_OFFS = [0, 64, 0, 64, 0, 64, 0, 64]
_WCOLS = 64 + _NNZ  # 113 padded stationary columns

# Ring balance: sync ring carries copyA + y stores (49*B descs); scalar ring
# carries the edge/weight loads (~1440 descs) + copyB. Split along k with
# 3-dim (b, k, c) APs — flattened (b k) APs break the even descriptor spray.
_KSPLIT = 578  # k rows 0:578 on the sync ring; rest on scalar

_cached = {}


def _weights():
    """Padded stationary weights, one set per example.

    wp (B, 128, WCOLS) over cells 896..1023 and w2p (B, 32, WCOLS) over cells
    0..31. For example b, column OFFS[b] + o corresponds to output cell
    k = 975 + o; entries are the neighbor coefficients scaled by
    factor[k] = 8^(k-1024) (exact in fp32); all other columns are zero.
    """
    t = np.arange(_K)
    factor = (np.float64(2.0) ** (3.0 * (t - _K))).astype(np.float32)
    w1 = np.zeros((128, _NNZ), np.float32)
    w2 = np.zeros((_N, _NNZ), np.float32)
    for o in range(_NNZ):
        k = _Y0 + o
        i, j = divmod(k, _N)
        f = factor[k]
        i1, i2 = (i + 1) % _N, (i - 2) % _N
        jp, jm = (j + 1) % _N, (j - 2) % _N
        for r, q in [(i1, jp), (i1, jm), (i2, jp), (i2, jm)]:
            cell = _N * r + q
            if cell >= _HI:
                w1[cell - _HI, o] += f
            else:
                w2[cell, o] += f
        w1[k - _HI, o] += np.float32(-4.0) * f
    wp = np.zeros((_B, 128, _WCOLS), np.float32)
    w2p = np.zeros((_B, _N, _WCOLS), np.float32)
    for b in range(_B):
        o = _OFFS[b]
        wp[b, :, o : o + _NNZ] = w1
        w2p[b, :, o : o + _NNZ] = w2
    return wp, w2p


def _build_nc():
    import concourse.bacc as bacc
    import concourse.mybir as mybir
    import concourse.tile as tile

    nc = bacc.Bacc("TRN2", debug=False, num_devices=_NCORES)
    f32 = mybir.dt.float32
    x_ap = nc.dram_tensor("x", (_B, _K, _C), f32, kind="ExternalInput").ap()
    xe1_ap = nc.dram_tensor("xe1", (_B, 128, _C), f32, kind="ExternalInput").ap()
    xe2_ap = nc.dram_tensor("xe2", (_B, _N, _C), f32, kind="ExternalInput").ap()
    wp_ap = nc.dram_tensor("wp", (128, _B * _WCOLS), f32, kind="ExternalInput").ap()
    w2p_ap = nc.dram_tensor("w2p", (_N, _B * _WCOLS), f32, kind="ExternalInput").ap()
    out_ap = nc.dram_tensor("out", (_B, _K, 2 * _C), f32, kind="ExternalOutput").ap()

    with tile.TileContext(nc) as tc, ExitStack() as ctx:
        pool = ctx.enter_context(tc.tile_pool(name="sbuf", bufs=1))
        psum_pool = ctx.enter_context(tc.tile_pool(name="psum", bufs=4, space="PSUM"))

        # Sync-ring half of the bulk copy goes first in that ring's FIFO.
        nc.sync.dma_start(
            out=out_ap[:, 0:_KSPLIT, 0:_C], in_=x_ap[:, 0:_KSPLIT, :]
        )

        # Scalar ring: weight + edge loads first, then its copy half.
        wp = pool.tile([128, _B * _WCOLS], f32, tag="wp")
        nc.scalar.dma_start(out=wp[:], in_=wp_ap)
        w2p = pool.tile([_N, _B * _WCOLS], f32, tag="w2p")
        nc.scalar.dma_start(out=w2p[:], in_=w2p_ap)

        x1s = []
        x2s = []
        for b in range(_B):
            x1 = pool.tile([128, _C], f32, tag=f"x1_{b}", name=f"x1_{b}")
            nc.scalar.dma_start(out=x1[:], in_=xe1_ap[b])
            x1s.append(x1)
            x2 = pool.tile([_N, _C], f32, tag=f"x2_{b}", name=f"x2_{b}")
            nc.scalar.dma_start(out=x2[:], in_=xe2_ap[b])
            x2s.append(x2)

        nc.scalar.dma_start(
            out=out_ap[:, _KSPLIT:_K, 0:_C], in_=x_ap[:, _KSPLIT:_K, :]
        )

        for b in range(_B):
            o = _OFFS[b]
            wsl = slice(b * _WCOLS, (b + 1) * _WCOLS)
            ps = psum_pool.tile([_WCOLS, _C], f32)
            nc.tensor.matmul(ps[:], wp[:, wsl], x1s[b][:], start=True, stop=False)
            nc.tensor.matmul(ps[:], w2p[:, wsl], x2s[b][:], start=False, stop=True)
            y = pool.tile([_WCOLS, _C], f32, tag=f"y_{b}", name=f"y_{b}")
            nc.vector.tensor_copy(y[o : o + _NNZ, :], ps[o : o + _NNZ, :])
            # Store rides the sync ring, behind copyA in its FIFO.
            nc.sync.dma_start(
                out=out_ap[b, _Y0:_K, _C : 2 * _C], in_=y[o : o + _NNZ, :]
            )

    nc.compile()
    return nc


def _get_nc():
    if "nc" not in _cached:
        _cached["nc"] = _build_nc()
    return _cached["nc"]


def _in_maps(x):
    wp, w2p = _weights()
    wpf = np.ascontiguousarray(wp.transpose(1, 0, 2).reshape(128, _B * _WCOLS))
    w2pf = np.ascontiguousarray(w2p.transpose(1, 0, 2).reshape(_N, _B * _WCOLS))
    maps = []
    for i in range(_NCORES):
        xs = np.ascontiguousarray(x[i * _B : (i + 1) * _B])
        maps.append(
            {
                "x": xs,
                "xe1": np.ascontiguousarray(xs[:, _HI:_K, :]),
                "xe2": np.ascontiguousarray(xs[:, 0:_N, :]),
                "wp": wpf,
                "w2p": w2pf,
            }
        )
    return maps


def kernel(x):
    from concourse.bass_utils import run_bass_kernel_spmd

    x = np.asarray(x, dtype=np.float32)
    assert x.shape == (_B_FULL, _K, _C), x.shape
    nc = _get_nc()
    res = run_bass_kernel_spmd(nc, _in_maps(x), list(range(_NCORES)))
    return np.concatenate([r["out"] for r in res.results], axis=0)


# revision 9
# speedup vs baseline: 1.6492x; 1.6447x over previous
"""Trainium2 Bass kernel for NeighborAggregation.

Math: for x of shape (b, k=1024, c=512) viewed as a 32x32 grid over k,
the reference computes y[cell t] = s(t) * 8^(t-1024) where s is a sum of 4
circularly-shifted neighbors minus 4x, and returns concat(x, y) on the c axis.
8^(t-1024) underflows to exactly 0.0 in fp32 for t <= 974, so y is nonzero
only for the last 49 k-rows (t = 975..1023), whose neighbor cells all live in
grid rows {0, 28..31} = flat cells [0..31] and [896..1023].

Kernel strategy (pure data parallel, batch 64 -> 8 cores x 8 examples):
  1. The bulk x -> out[:, :, 0:512] copy (16.78 MB/core, the dominant cost;
     every byte passes one SDMA engine once at ~25.6 GB/s/engine) is split
     across BOTH HWDGE rings (SP via nc.sync + ACT via nc.scalar), sized so
     the two ring FIFOs drain at the same time given the small transfers that
     share them.
  2. The 49 nonzero y rows per example come from a sparse fp32 matmul with
     the 8^(t-1024) factors folded into the weights. Alternate examples use
     stationary weights zero-padded by 64 leading columns, so their results
     land at PSUM/SBUF partitions 64..112 instead of 0..48. SBUF->HBM store
     descriptors are port-bound (engine = partition//8), so this spreads the
     store work over engines 0-6 AND 8-14 instead of piling on 0-6.
  3. Edge rows (cells 0..31 and 896..1023) and padded weights are uploaded
     pre-transposed/contiguous so loads are few large line-rate descriptors.
  4. The zero region of y is never written: ExternalOutput buffers are
     pre-zeroed by the runner.
"""

from contextlib import ExitStack

import numpy as np

_B_FULL, _K, _C = 64, 1024, 512
_NCORES = 8
_B = _B_FULL // _NCORES  # examples per core
_N = 32
_HI = 896  # first cell of grid rows 28..31
_NNZ = 49  # cells 975..1023 have nonzero factor
_Y0 = _K - _NNZ  # 975
# Per-example partition offset for the y result. A 49-partition DVE write
# must start quad-aligned AND stay within partitions 0-63 or 64-127 (bank
# straddle rule), so only offsets {0, 64} are legal.
_OFFS = [0, 64, 0, 64, 0, 64, 0, 64]
_WCOLS = 64 + _NNZ  # 113 padded stationary columns

# Ring balance: sync ring carries copyA + y stores; scalar ring carries the
# edge/weight loads + copyB. Each half's SOURCE AP must stay contiguous
# (collapsible to one flat dim) so the HWDGE sprays descriptors round-robin
# over all 16 SDMA engines -- a k-sliced source (outer dim 8) fans over only
# 8 engines. So split along b.
_BSPLIT = 4  # examples 0:4 on the sync ring; rest on scalar

_cached = {}


def _weights():
    """Padded stationary weights, one set per example.

    wp (B, 128, WCOLS) over cells 896..1023 and w2p (B, 32, WCOLS) over cells
    0..31. For example b, column OFFS[b] + o corresponds to output cell
    k = 975 + o; entries are the neighbor coefficients scaled by
    factor[k] = 8^(k-1024) (exact in fp32); all other columns are zero.
    """
    t = np.arange(_K)
    factor = (np.float64(2.0) ** (3.0 * (t - _K))).astype(np.float32)
    w1 = np.zeros((128, _NNZ), np.float32)
    w2 = np.zeros((_N, _NNZ), np.float32)
    for o in range(_NNZ):
        k = _Y0 + o
        i, j = divmod(k, _N)
        f = factor[k]
        i1, i2 = (i + 1) % _N, (i - 2) % _N
        jp, jm = (j + 1) % _N, (j - 2) % _N
        for r, q in [(i1, jp), (i1, jm), (i2, jp), (i2, jm)]:
            cell = _N * r + q
            if cell >= _HI:
                w1[cell - _HI, o] += f
            else:
                w2[cell, o] += f
        w1[k - _HI, o] += np.float32(-4.0) * f
    wp = np.zeros((_B, 128, _WCOLS), np.float32)
    w2p = np.zeros((_B, _N, _WCOLS), np.float32)
    for b in range(_B):
        o = _OFFS[b]
        wp[b, :, o : o + _NNZ] = w1
        w2p[b, :, o : o + _NNZ] = w2
    return wp, w2p


def _build_nc():
    import concourse.bacc as bacc
    import concourse.mybir as mybir
    import concourse.tile as tile

    nc = bacc.Bacc("TRN2", debug=False, num_devices=_NCORES)
    f32 = mybir.dt.float32
    x_ap = nc.dram_tensor("x", (_B, _K, _C), f32, kind="ExternalInput").ap()
    xe1_ap = nc.dram_tensor("xe1", (_B, 128, _C), f32, kind="ExternalInput").ap()
    xe2_ap = nc.dram_tensor("xe2", (_B, _N, _C), f32, kind="ExternalInput").ap()
    wp_ap = nc.dram_tensor("wp", (128, _B * _WCOLS), f32, kind="ExternalInput").ap()
    w2p_ap = nc.dram_tensor("w2p", (_N, _B * _WCOLS), f32, kind="ExternalInput").ap()
    out_ap = nc.dram_tensor("out", (_B, _K, 2 * _C), f32, kind="ExternalOutput").ap()

    with tile.TileContext(nc) as tc, ExitStack() as ctx:
        pool = ctx.enter_context(tc.tile_pool(name="sbuf", bufs=1))
        psum_pool = ctx.enter_context(tc.tile_pool(name="psum", bufs=4, space="PSUM"))

        # Sync-ring half of the bulk copy goes first in that ring's FIFO.
        nc.sync.dma_start(
            out=out_ap[0:_BSPLIT, :, 0:_C], in_=x_ap[0:_BSPLIT, :, :]
        )

        # Scalar ring: weight + edge loads first, then its copy half.
        wp = pool.tile([128, _B * _WCOLS], f32, tag="wp")
        nc.scalar.dma_start(out=wp[:], in_=wp_ap)
        w2p = pool.tile([_N, _B * _WCOLS], f32, tag="w2p")
        nc.scalar.dma_start(out=w2p[:], in_=w2p_ap)

        x1s = []
        x2s = []
        for b in range(_B):
            x1 = pool.tile([128, _C], f32, tag=f"x1_{b}", name=f"x1_{b}")
            nc.scalar.dma_start(out=x1[:], in_=xe1_ap[b])
            x1s.append(x1)
            x2 = pool.tile([_N, _C], f32, tag=f"x2_{b}", name=f"x2_{b}")
            nc.scalar.dma_start(out=x2[:], in_=xe2_ap[b])
            x2s.append(x2)

        nc.scalar.dma_start(
            out=out_ap[_BSPLIT:_B, :, 0:_C], in_=x_ap[_BSPLIT:_B, :, :]
        )

        for b in range(_B):
            o = _OFFS[b]
            wsl = slice(b * _WCOLS, (b + 1) * _WCOLS)
            ps = psum_pool.tile([_WCOLS, _C], f32)
            nc.tensor.matmul(ps[:], wp[:, wsl], x1s[b][:], start=True, stop=False)
            nc.tensor.matmul(ps[:], w2p[:, wsl], x2s[b][:], start=False, stop=True)
            y = pool.tile([_WCOLS, _C], f32, tag=f"y_{b}", name=f"y_{b}")
            nc.vector.tensor_copy(y[o : o + _NNZ, :], ps[o : o + _NNZ, :])
            # Store rides the sync ring, behind copyA in its FIFO.
            nc.sync.dma_start(
                out=out_ap[b, _Y0:_K, _C : 2 * _C], in_=y[o : o + _NNZ, :]
            )

    nc.compile()
    return nc


def _get_nc():
    if "nc" not in _cached:
        _cached["nc"] = _build_nc()
    return _cached["nc"]


def _in_maps(x):
    wp, w2p = _weights()
    wpf = np.ascontiguousarray(wp.transpose(1, 0, 2).reshape(128, _B * _WCOLS))
    w2pf = np.ascontiguousarray(w2p.transpose(1, 0, 2).reshape(_N, _B * _WCOLS))
    maps = []
    for i in range(_NCORES):
        xs = np.ascontiguousarray(x[i * _B : (i + 1) * _B])
        maps.append(
            {
                "x": xs,
                "xe1": np.ascontiguousarray(xs[:, _HI:_K, :]),
                "xe2": np.ascontiguousarray(xs[:, 0:_N, :]),
                "wp": wpf,
                "w2p": w2pf,
            }
        )
    return maps


def kernel(x):
    from concourse.bass_utils import run_bass_kernel_spmd

    x = np.asarray(x, dtype=np.float32)
    assert x.shape == (_B_FULL, _K, _C), x.shape
    nc = _get_nc()
    res = run_bass_kernel_spmd(nc, _in_maps(x), list(range(_NCORES)))
    return np.concatenate([r["out"] for r in res.results], axis=0)
